# revision 10
# baseline (speedup 1.0000x reference)
"""Differential multi-head attention on 8 Trainium2 NeuronCores.

Sharding: tensor-parallel over heads x data-parallel over batch.
Core c handles batch b = c//4 and real heads [4*(c%4), 4*(c%4)+4).
Each core computes a partial output (its 256 attention features through
the output projection); the host sums the 4 partials per batch.

Per-core dataflow (all matmuls bf16 with fp32 PSUM accumulation):
  qT/kT = W @ x.T          [feat, s] layout (feat on partitions)
  v     = x @ Wv.T         [s, feat] layout, plus a ones column per head
  ST_c  = k_c^T q_c        scores transposed: [keys, q] (keys on partitions)
  PT_c  = exp(ST_c)        on ScalarE; scores bounded ~6.5 so exp never
                           overflows and no max-subtraction is needed
  O_c   = PT_c^T @ v_aug   PV with P STATIONARY (full 128-wide loads) and
                           v_aug [keys,65] moving: out [q, 65] accumulates
                           over the 16 key tiles directly in PSUM; col 64
                           is the softmax denominator (ones-column trick).
                           This halves the billed PE time vs v-stationary
                           (65-col moving vs 256-col) and the output needs
                           no transpose before normalization.
  per-q: O = O1/r1 - lam*O2/r2; rms = exp(-0.5*ln(ssq/64 + eps));
  attn = O*rms (subln_w, 1-lam_init and the q scaling are folded into the
  weights on the host)
  out += attnT @ Wo'       partial over this core's 256 features

ScalarE is the bottleneck (~257us of exp): scores are built in
[128,1536] PSUM slabs (4/6/6 key tiles x 256 queries) so each exp
instruction covers 1024-1536 columns, amortizing the fixed per-
instruction SBUF-access overhead. The two components' slabs ping-pong
two PSUM buffers; PE (fills+PV+projections ~220us) hides behind the exp
stream. QKV projection rounds and the deferred per-chunk rms/output-
projection work are spread across early/later units so ScalarE never
waits.
"""

import math
import sys

sys.path.insert(0, "/opt/trn_rl_repo")

from contextlib import ExitStack

import ml_dtypes
import numpy as np

import concourse.bacc as bacc
import concourse.mybir as mybir
import concourse.tile as tile
from concourse.bass_utils import run_bass_kernel_spmd

# The kernel's only transcendentals are Exp and Ln; make the activation
# table-set chooser prefer the one set containing both, so a single
# ACT_TABLE_LOAD covers the whole kernel (the default order picks
# exp_and_others for Exp, forcing table reloads around the rms Ln).
_orig_get_activation_tables = bacc.get_activation_tables


def _tables_ln_exp_pinned(arch):
    # Keep dict ORDER identical (act_func_set_id is a positional index into
    # act_info.json), but remove Exp/Ln from every other set so the chooser
    # can only satisfy them from the combined set.
    t = dict(_orig_get_activation_tables(arch))
    pref = "natural_log_exp_and_others"
    if pref not in t:
        return t
    A = mybir.ActivationFunctionType
    out = {}
    for k, v in t.items():
        if k != pref:
            v = {f for f in v if f not in (A.Exp, A.Ln)}
        out[k] = v
    return out


bacc.get_activation_tables = _tables_ln_exp_pinned

F32 = mybir.dt.float32
BF16 = mybir.dt.bfloat16
ALU = mybir.AluOpType
ACT = mybir.ActivationFunctionType

E = 1024          # embed dim
S = 2048          # sequence length
B = 2             # batch
H = 16            # real heads
D = 32            # head dim (per component)
NCORES = 8
HPC = 4           # real heads per core
FPC = HPC * 2 * D  # features per core for q/k/v slices = 256
LAMBDA_INIT = 0.8 - 0.6 * math.exp(-0.3 * 12)
EPS = 1e-5

QC = 256          # query-chunk width
NQC = S // QC     # 8
NST = QC // 128   # q-subtiles per chunk
NKT = S // 128    # 16 key tiles
# key-tile slabs per component: exp instruction = one slab (cols = kt*QC)
SLABS = [(0, 4), (4, 10), (10, 16)]
NSLAB = len(SLABS)
OTW = 66          # ot column stride (65 cols + 1 pad for 8B alignment)


def build_kernel(reps: int = 1):
    nc = bacc.Bacc("TRN2", target_bir_lowering=False, debug=False,
                   num_devices=NCORES)
    xT = nc.dram_tensor("xT", [E, S], BF16, kind="ExternalInput")
    wq = nc.dram_tensor("wq", [E, FPC], BF16, kind="ExternalInput")
    wk = nc.dram_tensor("wk", [E, FPC], BF16, kind="ExternalInput")
    wv = nc.dram_tensor("wv", [E, FPC], BF16, kind="ExternalInput")
    wo = nc.dram_tensor("wo", [FPC, E], BF16, kind="ExternalInput")
    lam = nc.dram_tensor("lam", [128, 2], F32, kind="ExternalInput")
    idb = nc.dram_tensor("idb", [128, 128], BF16, kind="ExternalInput")
    out = nc.dram_tensor("out", [S, E], F32, kind="ExternalOutput")

    with tile.TileContext(nc) as tc, ExitStack() as ctx:
        cpool = ctx.enter_context(tc.tile_pool(name="consts", bufs=1))
        ipool = ctx.enter_context(tc.tile_pool(name="inputs", bufs=1))
        qpool = ctx.enter_context(tc.tile_pool(name="qkv", bufs=1))
        ptp = ctx.enter_context(tc.tile_pool(name="pt", bufs=2))
        apool = ctx.enter_context(tc.tile_pool(name="araw", bufs=1))
        wpool = ctx.enter_context(tc.tile_pool(name="work", bufs=3))
        ps_st = ctx.enter_context(tc.tile_pool(name="pst", bufs=2, space="PSUM"))
        ps_ot = ctx.enter_context(tc.tile_pool(name="pot", bufs=2, space="PSUM"))

        # Consolidated DMAs: every dma_start costs ~625ns on the single
        # serialized HWDGE descriptor engine, so the 8 row-blocks of each
        # weight live in ONE [128, 8*256] tile loaded by ONE transfer
        # (dram rows (kb p) -> tile cols (kb c)). x is split into 4
        # transfers on 4 different queues so its 32KB/partition overlaps.
        # Order matters: wk and x gate the first score fills; lam/idb/wo
        # are only needed mid-kernel.
        wk_all = ipool.tile([128, 8 * FPC], BF16, tag="wk")
        nc.sync.dma_start(
            wk_all.rearrange("p (kb c) -> p kb c", kb=8),
            wk.ap().rearrange("(kb p) c -> p kb c", kb=8))
        x_all = ipool.tile([128, 8 * S], BF16, tag="x")
        x_r = x_all.rearrange("p (kb s) -> p kb s", kb=8)
        xT_r = xT.ap().rearrange("(kb p) s -> p kb s", kb=8)
        for i, eng in enumerate((nc.sync, nc.scalar, nc.gpsimd, nc.sync)):
            eng.dma_start(x_r[:, 2 * i:2 * i + 2, :], xT_r[:, 2 * i:2 * i + 2, :])
        wq_all = ipool.tile([128, 8 * FPC], BF16, tag="wq")
        nc.sync.dma_start(
            wq_all.rearrange("p (kb c) -> p kb c", kb=8),
            wq.ap().rearrange("(kb p) c -> p kb c", kb=8))
        wv_all = ipool.tile([128, 8 * FPC], BF16, tag="wv")
        nc.sync.dma_start(
            wv_all.rearrange("p (kb c) -> p kb c", kb=8),
            wv.ap().rearrange("(kb p) c -> p kb c", kb=8))
        lamt = cpool.tile([128, 2], F32, tag="lam")
        nc.sync.dma_start(lamt[:], lam.ap())
        lam_sb = lamt[:, 0:1]
        eps_sb = lamt[:, 1:2]
        idb_sb = cpool.tile([128, 128], BF16, tag="idb")
        nc.sync.dma_start(idb_sb[:], idb.ap())
        wo_sb = []
        for fb in range(2):
            t = ipool.tile([128, E], BF16, tag=f"wo{fb}", name="t")
            nc.sync.dma_start(t[:], wo.ap()[fb * 128:(fb + 1) * 128, :])
            wo_sb.append(t)

        def wqk_slice(w_all, kb, fb):
            return w_all[:, kb * FPC + fb * 128:kb * FPC + (fb + 1) * 128]

        for _rep in range(reps):
            # ---------------- QKV projection rounds ----------------
            qt, kt = [None, None], [None, None]
            vt = [None] * NKT

            def proj_qk_round(dname, dst_list, w_all, fb, nch):
                if dst_list[fb] is None:
                    dst_list[fb] = qpool.tile([128, S], BF16,
                                              tag=f"{dname}{fb}", name="t")
                t = dst_list[fb]
                ps = ps_ot.tile([128, 512], F32, tag="pot")
                for kb in range(8):
                    nc.tensor.matmul(
                        ps[:], wqk_slice(w_all, kb, fb),
                        x_all[:, kb * S + nch * 512:kb * S + (nch + 1) * 512],
                        start=(kb == 0), stop=(kb == 7))
                nc.vector.tensor_copy(
                    t[:, nch * 512:(nch + 1) * 512], ps[:])

            def proj_v(st):
                t = qpool.tile([128, HPC * 65], BF16, tag=f"v{st}")
                vt[st] = t
                ps = ps_ot.tile([128, 512], F32, tag="pot")
                for kb in range(8):
                    nc.tensor.matmul(
                        ps[:, 0:FPC],
                        x_all[:, kb * S + st * 128:kb * S + (st + 1) * 128],
                        wv_all[:, kb * FPC:(kb + 1) * FPC],
                        start=(kb == 0), stop=(kb == 7))
                tv = t.rearrange("p (h x) -> p h x", x=65)
                nc.vector.tensor_copy(
                    tv[:, :, 0:64],
                    ps[:, 0:FPC].rearrange("p (h x) -> p h x", x=64))
                nc.vector.memset(tv[:, :, 64:65], 1.0)

            # ---------------- attention helpers ----------------
            def fill_slab(u, c, si):
                """Score matmuls for one (component, slab): [keys, q]."""
                k0, k1 = SLABS[si]
                stp = ps_st.tile([128, 1536], F32, tag="st")
                off = u["off1"] if c == 0 else u["off2"]
                tp = (off, 0) if off == 96 else None
                for j in range(k1 - k0):
                    ktile = k0 + j
                    nc.tensor.matmul(
                        stp[:, j * QC:(j + 1) * QC],
                        kt[u["fb"]][off:off + 32,
                                    ktile * 128:(ktile + 1) * 128],
                        qt[u["fb"]][off:off + 32,
                                    u["qc"] * QC:(u["qc"] + 1) * QC],
                        start=True, stop=True, tile_position=tp)
                return stp

            def exp_slab(u, c, si, stp):
                k0, k1 = SLABS[si]
                pt = u["pt1"] if c == 0 else u["pt2"]
                nc.scalar.activation(
                    pt[:, k0 * QC:k1 * QC], stp[:, 0:(k1 - k0) * QC],
                    ACT.Exp)

            def ensure_v(k0, k1):
                for st in range(k0, k1):
                    if vt[st] is None:
                        proj_v(st)

            def pv_slab(u, c, si):
                """PV for one (component, slab): P stationary, v moving.

                out[q,65] accumulates over key tiles in ONE psum bank; the
                very first matmul of the unit uses start=True (clears the
                bank's has_written bits), every other region's first write
                relies on still-clear bits (start=False overwrites).
                """
                k0, k1 = SLABS[si]
                ensure_v(k0, k1)
                pt = u["pt1"] if c == 0 else u["pt2"]
                h = u["h"]
                for st in range(NST):
                    col = OTW * (2 * st + c)
                    for j in range(k0, k1):
                        nc.tensor.matmul(
                            u["ot"][:, col:col + 65],
                            pt[:, j * QC + st * 128:j * QC + st * 128 + 128],
                            vt[j][:, h * 65:(h + 1) * 65],
                            start=(j == 0 and st == 0 and c == 0),
                            stop=(j == NKT - 1),
                            skip_group_check=True)

            def make_normalize(u):
                ot, h = u["ot"], u["h"]
                araw, ssq = u["araw"], u["ssq"]

                def _normalize():
                    for st in range(NST):
                        c1o = OTW * (2 * st)
                        c2o = OTW * (2 * st + 1)
                        inv1 = wpool.tile([128, 1], F32, tag="inv1")
                        inv2 = wpool.tile([128, 1], F32, tag="inv2")
                        nc.vector.reciprocal(inv1[:], ot[:, c1o + 64:c1o + 65])
                        nc.vector.reciprocal(inv2[:], ot[:, c2o + 64:c2o + 65])
                        o1n = wpool.tile([128, 64], F32, tag="o1n")
                        o2n = wpool.tile([128, 64], F32, tag="o2n")
                        nc.vector.tensor_scalar_mul(
                            o1n[:], ot[:, c1o:c1o + 64], inv1[:])
                        nc.vector.tensor_scalar(
                            o2n[:], ot[:, c2o:c2o + 64],
                            inv2[:], lam_sb, op0=ALU.mult, op1=ALU.mult)
                        nc.vector.tensor_sub(
                            araw[:, st, h, :], o1n[:], o2n[:])
                        sqs = wpool.tile([128, 64], F32, tag="sqs")
                        nc.vector.tensor_mul(
                            sqs[:], araw[:, st, h, :], araw[:, st, h, :])
                        nc.vector.tensor_reduce(
                            ssq[:, st * HPC + h:st * HPC + h + 1], sqs[:],
                            axis=mybir.AxisListType.X, op=ALU.add)
                return _normalize

            def make_rms(qc, araw, ssq, box):
                def _rms():
                    # rms scale = exp(-0.5 * ln(ssq/64 + eps))
                    rln = wpool.tile([128, NST * HPC], F32, tag="rln")
                    rmsi = wpool.tile([128, NST * HPC], F32, tag="rmsi")
                    nc.scalar.activation(rln[:], ssq[:], ACT.Ln,
                                         scale=1.0 / 64.0, bias=eps_sb)
                    nc.scalar.activation(rmsi[:], rln[:], ACT.Exp, scale=-0.5)
                    attn_bf = wpool.tile([128, NST, HPC, 64], BF16, tag="abf")
                    for st in range(NST):
                        for h in range(HPC):
                            nc.vector.tensor_scalar_mul(
                                attn_bf[:, st, h, :], araw[:, st, h, :],
                                rmsi[:, st * HPC + h:st * HPC + h + 1])
                    box.append(attn_bf)
                return _rms

            def make_proj(qc, st, box):
                def _proj():
                    attn_bf = box[0]
                    att_flat = attn_bf.rearrange("p s h d -> p s (h d)")
                    atps = ps_ot.tile([128, 256], BF16, tag="pot", name="atps")
                    nc.tensor.transpose(atps[:, 0:128],
                                        att_flat[:, st, 0:128], idb_sb[:])
                    nc.tensor.transpose(atps[:, 128:256],
                                        att_flat[:, st, 128:256], idb_sb[:])
                    at0 = wpool.tile([128, 128], BF16, tag="at0")
                    at1 = wpool.tile([128, 128], BF16, tag="at1")
                    nc.vector.tensor_copy(at0[:], atps[:, 0:128])
                    nc.vector.tensor_copy(at1[:], atps[:, 128:256])
                    row = (qc * NST + st) * 128
                    for ec in range(2):
                        ops = ps_ot.tile([128, 512], F32, tag="pot")
                        nc.tensor.matmul(
                            ops[:], at0[:],
                            wo_sb[0][:, ec * 512:(ec + 1) * 512],
                            start=True, stop=False)
                        nc.tensor.matmul(
                            ops[:], at1[:],
                            wo_sb[1][:, ec * 512:(ec + 1) * 512],
                            start=False, stop=True)
                        osb = wpool.tile([128, 512], F32, tag="osb")
                        nc.vector.tensor_copy(osb[:], ops[:])
                        nc.sync.dma_start(
                            out.ap()[row:row + 128,
                                     ec * 512:(ec + 1) * 512], osb[:])
                return _proj

            # ---------------- global slab pipeline ----------------
            # Slabs from all units form one stream. Per iteration g:
            # exp(g) [ActE pacing], one extra projection round, fill(g+1)
            # [keeps the score-psum ping-pong exactly one slab ahead],
            # deferred DVE/ActE/PE work due at g, then pv(g - PVLAG).
            # PVLAG=7 spreads each unit's v-tile needs and PV matmuls over
            # the following unit's exp runway; lag must stay < 8 so the
            # pt buffers (2 per component) are fully read before reuse.
            from collections import defaultdict, deque

            # Heads 0-1 (fb0) over all chunks first, then per-chunk heads
            # 2-3: fb1 q/k projections spread over the heads-0/1 runway.
            units = [(qc, h) for h in (0, 1) for qc in range(NQC)]
            units += [(qc, h) for qc in range(NQC) for h in (2, 3)]
            seq = [(c, si) for si in range(NSLAB) for c in range(2)]
            NSPU = len(seq)  # slabs per unit
            glist = [(ui, c, si) for ui in range(len(units))
                     for (c, si) in seq]
            PVLAG = 7

            qc_state = {}
            ustate = {}

            def get_unit(ui):
                if ui in ustate:
                    return ustate[ui]
                qc, h = units[ui]
                if qc not in qc_state:
                    qc_state[qc] = (
                        apool.tile([128, NST, HPC, 64], F32,
                                   tag=f"araw{qc}", name="araw"),
                        apool.tile([128, NST * HPC], F32,
                                   tag=f"ssq{qc}", name="ssq"))
                araw_t, ssq_t = qc_state[qc]
                u = {"qc": qc, "h": h, "fb": h // 2,
                     "off1": 64 * (h % 2), "off2": 64 * (h % 2) + 32,
                     "araw": araw_t, "ssq": ssq_t,
                     "pt1": None, "pt2": None, "ot": None}
                ustate[ui] = u
                return u

            extra = deque()
            extra.extend([("qk", "kt", kt, wk_all, 0, nch)
                          for nch in range(1, 4)])
            extra.extend([("v", st) for st in range(0, 4)])
            extra.extend([("v", st) for st in range(4, 7)])
            extra.append(("qk", "qt", qt, wq_all, 0, 1))
            extra.extend([("v", st) for st in range(7, 10)])
            extra.append(("qk", "qt", qt, wq_all, 0, 2))
            extra.extend([("v", st) for st in range(10, 13)])
            extra.append(("qk", "qt", qt, wq_all, 0, 3))
            extra.extend([("v", st) for st in range(13, 16)])
            extra.extend([("qk", "kt", kt, wk_all, 1, nch)
                          for nch in range(4)])
            extra.extend([("qk", "qt", qt, wq_all, 1, nch)
                          for nch in range(4)])

            def run_extra():
                while extra:
                    item = extra.popleft()
                    if item[0] == "v":
                        if vt[item[1]] is not None:
                            continue
                        proj_v(item[1])
                    else:
                        _, dname, dst, w_all, fb, nch = item
                        proj_qk_round(dname, dst, w_all, fb, nch)
                    return

            pending = defaultdict(list)

            def do_fill(g):
                ui, c, si = glist[g]
                return fill_slab(get_unit(ui), c, si)

            # Prologue: just enough to start the exp stream.
            proj_qk_round("kt", kt, wk_all, 0, 0)
            proj_qk_round("qt", qt, wq_all, 0, 0)
            st_tiles = {0: do_fill(0)}

            NG = len(glist)
            last_g = NG - 1 + PVLAG + 15
            for g in range(last_g + 1):
                if g < NG:
                    ui, c, si = glist[g]
                    u = get_unit(ui)
                    if u["pt1"] is None:
                        u["pt1"] = ptp.tile([128, NKT * QC], BF16,
                                            tag="pt1", name="pt1")
                        u["pt2"] = ptp.tile([128, NKT * QC], BF16,
                                            tag="pt2", name="pt2")
                    exp_slab(u, c, si, st_tiles.pop(g))
                    run_extra()
                if g + 1 < NG:
                    st_tiles[g + 1] = do_fill(g + 1)
                for fn in pending.pop(g, []):
                    fn()
                pg = g - PVLAG
                if 0 <= pg < NG:
                    ui, c, si = glist[pg]
                    u = get_unit(ui)
                    if u["ot"] is None:
                        u["ot"] = ps_ot.tile([128, OTW * 4], F32,
                                             tag="pot", name="ot")
                    pv_slab(u, c, si)
                    if (c, si) == seq[-1]:
                        pending[g + 1].append(make_normalize(u))
                        if u["h"] == HPC - 1:
                            qc = u["qc"]
                            araw_t, ssq_t = qc_state[qc]
                            box = []
                            pending[g + 3].append(
                                make_rms(qc, araw_t, ssq_t, box))
                            pending[g + 5].append(make_proj(qc, 0, box))
                            pending[g + 7].append(make_proj(qc, 1, box))
            for gk in sorted(pending):
                for fn in pending.pop(gk, []):
                    fn()
            qc_state.clear()
            ustate.clear()
    nc.compile()
    return nc


def _prep_core_inputs(inputs, core):
    x = np.asarray(inputs["x"], np.float32)
    Wq = np.asarray(inputs["Wq"], np.float32)
    Wk = np.asarray(inputs["Wk"], np.float32)
    Wv = np.asarray(inputs["Wv"], np.float32)
    Wo = np.asarray(inputs["Wo"], np.float32)
    subln_w = np.asarray(inputs["subln_w"], np.float32)
    b, hg = core // 4, core % 4
    sl = slice(FPC * hg, FPC * (hg + 1))
    bf = ml_dtypes.bfloat16
    scaling = D ** -0.5
    lam_full = float(
        np.exp(np.sum(np.asarray(inputs["lambda_q1"], np.float64)
                      * np.asarray(inputs["lambda_k1"], np.float64)))
        - np.exp(np.sum(np.asarray(inputs["lambda_q2"], np.float64)
                        * np.asarray(inputs["lambda_k2"], np.float64)))
        + LAMBDA_INIT)
    wo_scale = (np.tile(subln_w, HPC)[:, None] * (1.0 - LAMBDA_INIT))
    return {
        "xT": np.ascontiguousarray(x[b].T).astype(bf),
        "wq": np.ascontiguousarray(Wq[sl].T * scaling).astype(bf),
        "wk": np.ascontiguousarray(Wk[sl].T).astype(bf),
        "wv": np.ascontiguousarray(Wv[sl].T).astype(bf),
        "wo": np.ascontiguousarray(Wo[:, sl].T * wo_scale).astype(bf),
        "lam": np.stack([np.full(128, lam_full, np.float32),
                         np.full(128, EPS, np.float32)], axis=1),
        "idb": np.eye(128, dtype=ml_dtypes.bfloat16),
    }


_CACHED = {}


def _get_kernel(reps=1):
    if reps not in _CACHED:
        _CACHED[reps] = build_kernel(reps)
    return _CACHED[reps]


def run_on_cores(inputs, reps=1):
    nc = _get_kernel(reps)
    in_maps = [_prep_core_inputs(inputs, c) for c in range(NCORES)]
    res = run_bass_kernel_spmd(nc, in_maps, core_ids=list(range(NCORES)))
    return res


def kernel(**inputs) -> np.ndarray:
    res = run_on_cores(inputs)
    out = np.zeros((B, S, E), np.float32)
    for c in range(NCORES):
        out[c // 4] += res.results[c]["out"]
    return out
